# revision 19
# baseline (speedup 1.0000x reference)
"""Trainium2 Bass kernel for a 3-layer GCN corrector (AccessibilityGNNCorrector).

Node-parallel across 8 NeuronCores (12500 dst nodes per core), fp16 data path.

Per GCN layer each core builds its fp16 shard of the gather table (rows
pre-scaled by dinv[src]; pad rows zero), AllGathers the full [100352, 128]
table, then aggregates its incoming edges with dma_gather (256B rows, bucketed
into 4 sections of 25088 table rows so indices fit int16) + one-hot selection
matmuls into PSUM.  The per-edge norm dinv[src]*dinv[dst] is split: src factor
baked into the table, dst factor applied once per dst tile, so the selection
matrix is a pure is_equal one-hot built in a single DVE op.  b1/b2 are dropped
(absorbed by BatchNorm).  Layer 3 gathers h2 directly and applies W3 after
aggregation (linearity), keeping all gather rows 256B.
BatchNorm statistics are AllReduced ([128,2] per layer).
"""

import os

import numpy as np

import concourse.bacc as bacc
import concourse.bass as bass
import concourse.mybir as mybir
import concourse.tile as tile
from concourse import library_config
from concourse.bass_utils import run_bass_kernel_spmd

F32 = mybir.dt.float32
F16 = mybir.dt.float16
I16 = mybir.dt.int16
AF = mybir.ActivationFunctionType
ALU = mybir.AluOpType

N_NODES = 100000
N_CORES = 8
SH = N_NODES // N_CORES          # 12500
TILE = 128
N_TILES = (SH + TILE - 1) // TILE         # 98
SH_PAD = N_TILES * TILE                   # 12544
N_SEC = 4
SEC_ROWS = SH_PAD * (N_CORES // N_SEC)    # 25088 table rows per section
PAD_IDX = SH                              # section-local zero row
T_MERGE = 4
N_GROUPS = (N_TILES + T_MERGE - 1) // T_MERGE   # 25
APPLY_W = 512
HID = 128
F3 = 64
ABLATE = os.environ.get("KABLATE", "")
PHASES = {"proj": 1, "ag1": 2, "agg1": 3, "l1": 4, "agg2": 5, "l2": 6,
          "agg3": 7, "full": 8}
PH = PHASES[os.environ.get("KPHASE", "full")]


# --------------------------------------------------------------------------
# host-side graph partitioning / metadata layout
# --------------------------------------------------------------------------

def _layout(cnt_max):
    """cnt_max: [N_TILES, N_SEC] padded (128-mult) token counts.

    Returns groups list; token positions global over concatenated calls.
    Each group: {tiles, chunk_base, n_chunks, calls:[{s, K, token_base,
    secs:[(t, n_chunks, chunk_off)]}]}.
    """
    groups = []
    tok = 0
    for g in range(N_GROUPS):
        tiles = list(range(g * T_MERGE, min((g + 1) * T_MERGE, N_TILES)))
        grp = {"tiles": tiles, "calls": [], "chunk_base": tok // TILE}
        for s in range(N_SEC):
            K = int(sum(cnt_max[t][s] for t in tiles))
            if K == 0:
                continue
            call = {"s": s, "K": K, "token_base": tok, "secs": []}
            off = 0
            for t in tiles:
                n_ch = cnt_max[t][s] // TILE
                if n_ch:
                    call["secs"].append((t, n_ch, off))
                off += n_ch
            grp["calls"].append(call)
            tok += K
        grp["n_chunks"] = tok // TILE - grp["chunk_base"]
        groups.append(grp)
    return groups, tok


def _prepare(x, edge_index):
    n = x.shape[0]
    assert n == N_NODES and x.shape[1] == 256

    src = np.ascontiguousarray(edge_index[0]).astype(np.int64)
    dst = np.ascontiguousarray(edge_index[1]).astype(np.int64)
    deg = (np.bincount(dst, minlength=n) + 1.0).astype(np.float64)
    dinv = (1.0 / np.sqrt(deg)).astype(np.float32)

    loops = np.arange(n, dtype=np.int64)
    src_a = np.concatenate([src, loops])
    dst_a = np.concatenate([dst, loops])

    # padded table row of each source + section bucketing
    srow = (src_a // SH) * SH_PAD + (src_a % SH)
    sec_a = srow // SEC_ROWS
    sid_a = (srow % SEC_ROWS).astype(np.int16)

    core_of = dst_a // SH
    per_core = []
    cnts = np.zeros((N_CORES, N_TILES, N_SEC), np.int64)
    for k in range(N_CORES):
        m = core_of == k
        d = dst_a[m] - k * SH
        s, sid = sec_a[m], sid_a[m]
        t = d >> 7
        order = np.lexsort((sid, s, t))
        d, s, sid, t = d[order], s[order], sid[order], t[order]
        np.add.at(cnts[k], (t, s), 1)
        per_core.append((d, s, sid, t))

    cnt_max = cnts.max(axis=0)
    cnt_pad = ((cnt_max + TILE - 1) // TILE * TILE).astype(np.int64)
    groups, total_tok = _layout(cnt_pad)
    total_chunks = total_tok // TILE

    sec_base = np.zeros((N_TILES, N_SEC), np.int64)
    for grp in groups:
        for call in grp["calls"]:
            for (t, n_ch, off) in call["secs"]:
                sec_base[t][call["s"]] = call["token_base"] + off * TILE

    idx_cols = total_tok // 16
    core_inputs = []
    for k in range(N_CORES):
        d, s, sid, t = per_core[k]
        flat_cnt = cnts[k].reshape(-1)
        sec_start = np.concatenate([[0], np.cumsum(flat_cnt)])[:-1]
        sec_id = t * N_SEC + s
        rank = np.arange(len(d)) - sec_start[sec_id]
        dest = sec_base.reshape(-1)[sec_id] + rank

        tok_idx = np.full(total_tok, PAD_IDX, np.int16)
        tok_dloc = np.zeros(total_tok, np.float16)
        tok_idx[dest] = sid
        tok_dloc[dest] = (d & (TILE - 1)).astype(np.float16)

        idx_arr = np.zeros((128, idx_cols), np.int16)
        for grp in groups:
            for call in grp["calls"]:
                a, K = call["token_base"], call["K"]
                wrp = tok_idx[a:a + K].reshape(-1, 16).T
                idx_arr[:, a // 16:(a + K) // 16] = np.tile(wrp, (8, 1))
        dloc_arr = np.ascontiguousarray(
            tok_dloc.reshape(total_chunks, TILE).T)

        xT = np.zeros((256, SH_PAD), np.float16)
        xT[:, :SH] = x[k * SH:(k + 1) * SH].T.astype(np.float16)
        dinvT = np.zeros((128, SH_PAD), np.float16)
        dinvT[:, :SH] = dinv[k * SH:(k + 1) * SH][None, :].astype(np.float16)
        core_inputs.append({"xT": xT, "idx": idx_arr, "dloc": dloc_arr,
                            "dinvT": dinvT})

    plan = {"groups": groups, "total_tok": total_tok,
            "total_chunks": total_chunks, "idx_cols": idx_cols}
    return core_inputs, plan


def _weight_inputs(inputs):
    smalls = np.zeros((128, 8), np.float32)
    smalls[:HID, 0] = inputs["bp"]
    smalls[:HID, 1] = inputs["gamma1"]
    smalls[:HID, 2] = inputs["beta1"]
    smalls[:HID, 3] = inputs["gamma2"]
    smalls[:HID, 4] = inputs["beta2"]
    smalls[:F3, 5] = inputs["b3"]
    smalls[0, 6] = inputs["bh"][0]
    smalls[:, 7] = 1e-5
    wp = np.asarray(inputs["Wp"], np.float16)
    return {
        "Wpa": np.ascontiguousarray(wp[:128]),
        "Wpb": np.ascontiguousarray(wp[128:256]),
        "W1": np.asarray(inputs["W1"], np.float16),
        "W2": np.asarray(inputs["W2"], np.float16),
        "W3": np.asarray(inputs["W3"], np.float16),
        "Wh": np.asarray(inputs["Wh"], np.float16),
        "smalls": smalls,
    }


# --------------------------------------------------------------------------
# device program
# --------------------------------------------------------------------------

def _apply_chunks():
    out = []
    c = 0
    while c < SH_PAD:
        w = min(APPLY_W, SH_PAD - c)
        out.append((c, w))
        c += w
    return out


def build_program(plan, scale_const):
    groups = plan["groups"]
    rg = [list(range(N_CORES))]
    inv_n = 1.0 / N_NODES

    nc = bacc.Bacc("TRN2", target_bir_lowering=False, debug=False,
                   num_devices=N_CORES, num_swdge_queues=4)

    # I/O
    xT_d = nc.dram_tensor("xT", [256, SH_PAD], F16, kind="ExternalInput")
    idx_d = nc.dram_tensor("idx", [128, plan["idx_cols"]], I16,
                           kind="ExternalInput")
    dloc_d = nc.dram_tensor("dloc", [128, plan["total_chunks"]], F16,
                            kind="ExternalInput")
    dinvT_d = nc.dram_tensor("dinvT", [128, SH_PAD], F16,
                             kind="ExternalInput")
    Wpa_d = nc.dram_tensor("Wpa", [128, HID], F16, kind="ExternalInput")
    Wpb_d = nc.dram_tensor("Wpb", [128, HID], F16, kind="ExternalInput")
    W1_d = nc.dram_tensor("W1", [HID, HID], F16, kind="ExternalInput")
    W2_d = nc.dram_tensor("W2", [HID, HID], F16, kind="ExternalInput")
    W3_d = nc.dram_tensor("W3", [HID, F3], F16, kind="ExternalInput")
    Wh_d = nc.dram_tensor("Wh", [F3, 1], F16, kind="ExternalInput")
    smalls_d = nc.dram_tensor("smalls", [128, 8], F32, kind="ExternalInput")
    out_d = nc.dram_tensor("out", [1, SH], F32, kind="ExternalOutput")

    # internal scratch
    hw_shard = [nc.dram_tensor(f"hws{i}", [SH_PAD, HID], F16)
                for i in range(3)]
    hw_full = [nc.dram_tensor(f"hwf{i}", [SH_PAD * N_CORES, HID], F16)
               for i in range(3)]
    stl_d = [nc.dram_tensor(f"stl{i}", [128, 2], F32) for i in range(2)]
    stg_d = [nc.dram_tensor(f"stg{i}", [128, 2], F32, addr_space="Shared")
             for i in range(2)]

    with tile.TileContext(nc) as tc:
        with (
            tc.tile_pool(name="const", bufs=1) as constp,
            tc.tile_pool(name="gbuf", bufs=6) as gpool,
            tc.tile_pool(name="big", bufs=3) as bigpool,
            tc.tile_pool(name="spool", bufs=8) as spool,
            tc.tile_pool(name="meta", bufs=3) as metapool,
            tc.tile_pool(name="small", bufs=4) as smallpool,
            tc.tile_pool(name="head", bufs=1) as headpool,
            tc.tile_pool(name="psA", bufs=2, space="PSUM") as psA,
            tc.tile_pool(name="psB", bufs=3, space="PSUM") as psB,
            tc.tile_pool(name="psT", bufs=2, space="PSUM") as psT,
            tc.tile_pool(name="psH", bufs=1, space="PSUM") as psH,
        ):
            nc.gpsimd.load_library(library_config.mlp)

            # ---- constants ----
            iota = constp.tile([128, 128], F32, tag="iota", name="iota")
            nc.gpsimd.iota(iota[:], pattern=[[1, 128]], base=0,
                           channel_multiplier=0,
                           allow_small_or_imprecise_dtypes=True)
            iota_c = constp.tile([128, 1], F32, tag="iota_c", name="iota_c")
            nc.gpsimd.iota(iota_c[:], pattern=[[0, 1]], base=0,
                           channel_multiplier=1,
                           allow_small_or_imprecise_dtypes=True)
            iota16 = constp.tile([128, 128], F16, tag="iota16", name="iota16")
            nc.scalar.copy(iota16[:], iota[:])
            eye16 = constp.tile([128, 128], F16, tag="eye16", name="eye16")
            nc.vector.tensor_scalar(eye16[:], iota[:], iota_c[:], None,
                                    ALU.is_equal)

            def load_const(name, dram, shape, dt):
                t = constp.tile(shape, dt, tag=name)
                nc.sync.dma_start(t[:], dram[:])
                return t

            Wpa = load_const("Wpa", Wpa_d, [128, HID], F16)
            Wpb = load_const("Wpb", Wpb_d, [128, HID], F16)
            W1 = load_const("W1", W1_d, [HID, HID], F16)
            W2 = load_const("W2", W2_d, [HID, HID], F16)
            W3 = load_const("W3", W3_d, [HID, F3], F16)
            Wh = load_const("Wh", Wh_d, [F3, 1], F16)
            smalls = load_const("smalls", smalls_d, [128, 8], F32)
            dinvT = load_const("dinvT", dinvT_d, [128, SH_PAD], F16)
            dlocs = load_const("dloc", dloc_d, [128, plan["total_chunks"]],
                               F16)

            # big SBUF slabs (persistent)
            aggs = constp.tile([128, SH_PAD], F16, tag="aggs", name="aggs")
            h1T = constp.tile([128, SH_PAD], F16, tag="h1T", name="h1T")

            stats_sum = [constp.tile([128, N_TILES], F32, tag=f"ssum{i}",
                                     name=f"ssum{i}") for i in (0, 1)]
            stats_sq = [constp.tile([128, N_TILES], F32, tag=f"ssq{i}",
                                    name=f"ssq{i}") for i in (0, 1)]

            # ---- helper: emit row-major fp16 table rows from T-land chunk --
            def emit_rows(src_tile, c0, w, shard):
                # src_tile: [128, w] fp16 T-land (features x nodes)
                for b0 in range(0, w, 512):
                    bw = min(512, w - b0)
                    nj = bw // 128
                    pst = psT.tile([128, 512], F16, tag="pst", name="pst")
                    for j in range(nj):
                        nc.tensor.transpose(
                            pst[:, j * 128:(j + 1) * 128],
                            src_tile[:, b0 + j * 128:b0 + (j + 1) * 128],
                            eye16[:])
                    rsb = bigpool.tile([128, 512], F16, tag="row", name="row")
                    nc.scalar.copy(rsb[:, :nj * 128], pst[:, :nj * 128])
                    r0 = c0 + b0
                    dview = shard[r0:r0 + nj * 128, :].rearrange(
                        "(j p) f -> p j f", p=128)
                    sview = rsb[:, :nj * 128].rearrange(
                        "p (j f) -> p j f", f=128)
                    nc.sync.dma_start(dview, sview)

            # ---- stage 0: projection + table 1 ----
            for (c0, w) in _apply_chunks():
                xa = bigpool.tile([128, APPLY_W], F16, tag="xa", name="xa")
                nc.sync.dma_start(xa[:, :w], xT_d[0:128, c0:c0 + w])
                xb = bigpool.tile([128, APPLY_W], F16, tag="xb", name="xb")
                nc.sync.dma_start(xb[:, :w], xT_d[128:256, c0:c0 + w])
                ps = psB.tile([128, APPLY_W], F32, tag="psb", name="psb")
                nc.tensor.matmul(ps[:, :w], Wpa[:], xa[:, :w],
                                 start=True, stop=False)
                nc.tensor.matmul(ps[:, :w], Wpb[:], xb[:, :w],
                                 start=False, stop=True)
                h0 = bigpool.tile([128, APPLY_W], F16, tag="hsb", name="hsb")
                nc.scalar.activation(h0[:, :w], ps[:, :w], AF.Relu,
                                     bias=smalls[:, 0:1])
                ht = bigpool.tile([128, APPLY_W], F16, tag="ht", name="ht")
                nc.vector.tensor_tensor(ht[:, :w], h0[:, :w],
                                        dinvT[:, c0:c0 + w], ALU.mult)
                ps2 = psB.tile([128, APPLY_W], F32, tag="psb", name="psb")
                nc.tensor.matmul(ps2[:, :w], W1[:], ht[:, :w],
                                 start=True, stop=True)
                hw1 = bigpool.tile([128, APPLY_W], F16, tag="hw", name="hw")
                nc.scalar.copy(hw1[:, :w], ps2[:, :w])
                emit_rows(hw1, c0, w, hw_shard[0])

            if ABLATE != "noag" and PH >= 2:
                nc.gpsimd.collective_compute(
                    "AllGather", ALU.bypass, replica_groups=rg,
                    ins=[hw_shard[0][:]], outs=[hw_full[0][:]])

            # ---- aggregation pass (layers 1..3) ----
            def agg_pass(li):
                table = hw_full[li - 1]
                for grp in groups:
                    if not grp["calls"]:
                        continue
                    ic0 = grp["calls"][0]["token_base"] // 16
                    icw = sum(c["K"] for c in grp["calls"]) // 16
                    islab = metapool.tile([128, max(icw, 1)], I16,
                                          tag="islab", name="islab")
                    nc.sync.dma_start(islab[:, :icw], idx_d[:, ic0:ic0 + icw])
                    gts = {}
                    for call in grp["calls"]:
                        K, s = call["K"], call["s"]
                        gt = gpool.tile([128, K // TILE, HID], F16,
                                        tag="g", name="g")
                        lo = (call["token_base"]
                              - grp["calls"][0]["token_base"]) // 16
                        assert K <= 8192, K
                        if ABLATE == "nogather":
                            gts[s] = (gt, call)
                            continue
                        nc.gpsimd.dma_gather(
                            gt[:], table[s * SEC_ROWS:(s + 1) * SEC_ROWS, :],
                            islab[:, lo:lo + K // 16],
                            K, K, HID, single_packet=False, queue_num=s)
                        gts[s] = (gt, call)
                    for t in grp["tiles"]:
                        runs = []
                        for s in sorted(gts):
                            gt, call = gts[s]
                            for (tt, n_ch, off) in call["secs"]:
                                if tt != t:
                                    continue
                                mc0 = call["token_base"] // TILE + off
                                for c0r in range(0, n_ch, 4):
                                    nr = min(4, n_ch - c0r)
                                    runs.append((gt, off + c0r, mc0 + c0r,
                                                 nr))
                        ps = psA.tile([HID, 128], F32, tag="psa", name="psa")
                        if ABLATE == "nomm":
                            runs = runs[:1]
                        n_mm = sum(r[3] for r in runs)
                        i = 0
                        for (gt, gc0, mc0, nr) in runs:
                            S4 = spool.tile([128, 512], F16, tag="S",
                                            name="S4")
                            dv = dlocs[:, mc0:mc0 + nr]
                            dX = bass.AP(dv.tensor, dv.offset,
                                         [list(dv.ap[0]), list(dv.ap[1]),
                                          [0, 128]])
                            iv = iota16[:]
                            iR = bass.AP(iv.tensor, iv.offset,
                                         [list(iv.ap[0]), [0, nr],
                                          list(iv.ap[1])])
                            nc.vector.tensor_tensor(
                                S4[:, :nr * 128].rearrange(
                                    "p (c f) -> p c f", f=128),
                                dX, iR, ALU.is_equal)
                            for j in range(nr):
                                nc.tensor.matmul(
                                    ps[:], gt[:, gc0 + j, :],
                                    S4[:, j * 128:(j + 1) * 128],
                                    start=(i == 0), stop=(i == n_mm - 1))
                                i += 1
                        yield t, ps

            # ---- layers 1 and 2 ----
            for li in (1, 2):
                if PH < (3 if li == 1 else 5):
                    break
                ssum, ssq = stats_sum[li - 1], stats_sq[li - 1]
                for t, ps in agg_pass(li):
                    col = aggs[:, t * 128:(t + 1) * 128]
                    nc.vector.tensor_tensor(col, ps[:],
                                            dinvT[:, t * 128:(t + 1) * 128],
                                            ALU.mult)
                    nc.vector.tensor_reduce(ssum[:, t:t + 1], col,
                                            mybir.AxisListType.X, ALU.add)
                    sq = smallpool.tile([128, 128], F32, tag="sq", name="sq")
                    nc.scalar.square(sq[:], col)
                    nc.vector.tensor_reduce(ssq[:, t:t + 1], sq[:],
                                            mybir.AxisListType.X, ALU.add)
                # stats -> AllReduce
                if PH < (4 if li == 1 else 6):
                    break
                st = smallpool.tile([128, 2], F32, tag="stp", name="stp")
                nc.vector.tensor_reduce(st[:, 0:1], ssum[:],
                                        mybir.AxisListType.X, ALU.add)
                nc.vector.tensor_reduce(st[:, 1:2], ssq[:],
                                        mybir.AxisListType.X, ALU.add)
                nc.sync.dma_start(stl_d[li - 1][:], st[:])
                if ABLATE != "noag":
                    nc.gpsimd.collective_compute(
                        "AllReduce", ALU.add, replica_groups=rg,
                        ins=[stl_d[li - 1][:]], outs=[stg_d[li - 1][:]])
                ssb = smallpool.tile([128, 2], F32, tag="ssb", name="ssb")
                nc.sync.dma_start(ssb[:], (stl_d if ABLATE == "noag" else
                                           stg_d)[li - 1][:])

                # BN scale/bias (f32), then cast fp16
                scr = smallpool.tile([128, 6], F32, tag="bnscr", name="bnscr")
                nc.vector.tensor_scalar(scr[:, 0:1], ssb[:, 0:1], inv_n,
                                        None, ALU.mult)              # mean
                nc.vector.tensor_scalar(scr[:, 1:2], ssb[:, 1:2], inv_n,
                                        None, ALU.mult)              # E[x^2]
                nc.vector.tensor_tensor(scr[:, 2:3], scr[:, 0:1],
                                        scr[:, 0:1], ALU.mult)       # mean^2
                nc.vector.tensor_tensor(scr[:, 3:4], scr[:, 1:2],
                                        scr[:, 2:3], ALU.subtract)   # var
                nc.scalar.activation(scr[:, 4:5], scr[:, 3:4], AF.Sqrt,
                                     bias=smalls[:, 7:8])            # std
                inv_t = smallpool.tile([128, 1], F32, tag="invs", name="invs")
                nc.vector.reciprocal(inv_t[:], scr[:, 4:5])
                gamma_ap = smalls[:, 2 * li - 1:2 * li]
                beta_ap = smalls[:, 2 * li:2 * li + 1]
                scale_f = smallpool.tile([128, 1], F32, tag="scf", name="scf")
                bias_f = smallpool.tile([128, 1], F32, tag="bif", name="bif")
                nc.vector.tensor_tensor(scale_f[:], inv_t[:], gamma_ap,
                                        ALU.mult)
                mb = smallpool.tile([128, 1], F32, tag="mb", name="mb")
                nc.vector.tensor_tensor(mb[:], scr[:, 0:1], scale_f[:],
                                        ALU.mult)
                nc.vector.tensor_tensor(bias_f[:], beta_ap, mb[:],
                                        ALU.subtract)

                # BN apply + next table build
                for (c0, w) in _apply_chunks():
                    if li == 1:
                        htg = h1T[:, c0:c0 + w]
                        nc.scalar.activation(htg, aggs[:, c0:c0 + w],
                                             AF.Relu, bias=bias_f[:],
                                             scale=scale_f[:])
                    else:
                        tmp = bigpool.tile([128, APPLY_W], F16, tag="hsb",
                                           name="hsb")
                        nc.scalar.activation(tmp[:, :w], aggs[:, c0:c0 + w],
                                             AF.Relu, bias=bias_f[:],
                                             scale=scale_f[:])
                        htg = h1T[:, c0:c0 + w]
                        nc.vector.tensor_tensor(htg, tmp[:, :w], htg,
                                                ALU.add)
                    ht = bigpool.tile([128, APPLY_W], F16, tag="ht",
                                      name="ht")
                    nc.vector.tensor_tensor(ht[:, :w], htg,
                                            dinvT[:, c0:c0 + w], ALU.mult)
                    if li == 1:
                        ps2 = psB.tile([128, APPLY_W], F32, tag="psb",
                                       name="psb")
                        nc.tensor.matmul(ps2[:, :w], W2[:], ht[:, :w],
                                         start=True, stop=True)
                        hw2 = bigpool.tile([128, APPLY_W], F16, tag="hw",
                                           name="hw")
                        nc.scalar.copy(hw2[:, :w], ps2[:, :w])
                        emit_rows(hw2, c0, w, hw_shard[1])
                    else:
                        emit_rows(ht, c0, w, hw_shard[2])
                if ABLATE != "noag":
                    nc.gpsimd.collective_compute(
                        "AllGather", ALU.bypass, replica_groups=rg,
                        ins=[hw_shard[li][:]], outs=[hw_full[li][:]])

            # ---- layer 3 + head ----
            for t, ps in (agg_pass(3) if PH >= 7 else ()):
                col = aggs[:, t * 128:(t + 1) * 128]
                nc.vector.tensor_tensor(col, ps[:],
                                        dinvT[:, t * 128:(t + 1) * 128],
                                        ALU.mult)
            b3_ap = smalls[:F3, 5:6]
            bh_ap = smalls[0:1, 6:7]
            for (c0, w) in (_apply_chunks() if PH >= 8 else ()):
                ps3 = psB.tile([F3, APPLY_W], F32, tag="psb", name="ps3")
                nc.tensor.matmul(ps3[:, :w], W3[:], aggs[:, c0:c0 + w],
                                 start=True, stop=True)
                h3 = bigpool.tile([F3, APPLY_W], F16, tag="h3", name="h3")
                nc.scalar.activation(h3[:, :w], ps3[:, :w], AF.Relu,
                                     bias=b3_ap)
                psh = psH.tile([1, APPLY_W], F32, tag="psh", name="psh")
                nc.tensor.matmul(psh[:, :w], Wh[:], h3[:, :w],
                                 start=True, stop=True)
                th = headpool.tile([1, APPLY_W], F32, tag="th", name="th")
                nc.scalar.activation(th[:, :w], psh[:, :w], AF.Tanh,
                                     bias=bh_ap)
                ow = headpool.tile([1, APPLY_W], F32, tag="ow", name="ow")
                nc.scalar.mul(ow[:, :w], th[:, :w], float(scale_const))
                w_out = min(w, SH - c0)
                if w_out > 0:
                    nc.sync.dma_start(out_d[0:1, c0:c0 + w_out],
                                      ow[0:1, :w_out])
            if PH < 8:
                nc.sync.dma_start(out_d[0:1, 0:8], smalls[0:1, :])

    nc.compile()
    return nc


# --------------------------------------------------------------------------
# entry point
# --------------------------------------------------------------------------

_CACHE = {}


def _build_all(inputs, n_cores=8):
    x = np.asarray(inputs["x"], np.float32)
    ei = np.asarray(inputs["edge_index"])
    core_inputs, plan = _prepare(x, ei)
    wmap = _weight_inputs(inputs)
    in_maps = [{**ci, **wmap} for ci in core_inputs]
    key = (x.shape, ei.shape, float(np.asarray(inputs["scale"])))
    if key in _CACHE:
        nc = _CACHE[key]
    else:
        nc = build_program(plan, float(np.asarray(inputs["scale"])))
        _CACHE[key] = nc
    return nc, in_maps, plan


def kernel(**inputs) -> np.ndarray:
    nc, in_maps, plan = _build_all(inputs, N_CORES)
    res = run_bass_kernel_spmd(nc, in_maps, list(range(N_CORES)))
    outs = [res.results[k]["out"].reshape(-1) for k in range(N_CORES)]
    return np.concatenate(outs).reshape(-1, 1).astype(np.float32)


# revision 20
# speedup vs baseline: 1.0248x; 1.0248x over previous
"""Trainium2 Bass kernel for a 3-layer GCN corrector (AccessibilityGNNCorrector).

Node-parallel across 8 NeuronCores (12500 dst nodes per core), fp16 data path.

Per GCN layer each core builds its fp16 shard of the gather table (rows
pre-scaled by dinv[src]; pad rows zero), AllGathers the full [100352, 128]
table, then aggregates its incoming edges with dma_gather (256B rows, bucketed
into 4 sections of 25088 table rows so indices fit int16) + one-hot selection
matmuls into PSUM.  The per-edge norm dinv[src]*dinv[dst] is split: src factor
baked into the table, dst factor applied once per dst tile, so the selection
matrix is a pure is_equal one-hot built in a single DVE op.  b1/b2 are dropped
(absorbed by BatchNorm).  Layer 3 gathers h2 directly and applies W3 after
aggregation (linearity), keeping all gather rows 256B.
BatchNorm statistics are AllReduced ([128,2] per layer).
"""

import os

import numpy as np

import concourse.bacc as bacc
import concourse.bass as bass
import concourse.mybir as mybir
import concourse.tile as tile
from concourse import library_config
from concourse.bass_utils import run_bass_kernel_spmd

F32 = mybir.dt.float32
F16 = mybir.dt.float16
I16 = mybir.dt.int16
AF = mybir.ActivationFunctionType
ALU = mybir.AluOpType

N_NODES = 100000
N_CORES = 8
SH = N_NODES // N_CORES          # 12500
TILE = 128
N_TILES = (SH + TILE - 1) // TILE         # 98
SH_PAD = N_TILES * TILE                   # 12544
N_SEC = 4
SEC_ROWS = SH_PAD * (N_CORES // N_SEC)    # 25088 table rows per section
PAD_IDX = SH                              # section-local zero row
T_MERGE = 4
N_GROUPS = (N_TILES + T_MERGE - 1) // T_MERGE   # 25
APPLY_W = 512
HID = 128
F3 = 64
ABLATE = os.environ.get("KABLATE", "")
PHASES = {"proj": 1, "ag1": 2, "agg1": 3, "l1": 4, "agg2": 5, "l2": 6,
          "agg3": 7, "full": 8}
PH = PHASES[os.environ.get("KPHASE", "full")]


# --------------------------------------------------------------------------
# host-side graph partitioning / metadata layout
# --------------------------------------------------------------------------

def _layout(cnt_max):
    """cnt_max: [N_TILES, N_SEC] padded (128-mult) token counts.

    Returns groups list; token positions global over concatenated calls.
    Each group: {tiles, chunk_base, n_chunks, calls:[{s, K, token_base,
    secs:[(t, n_chunks, chunk_off)]}]}.
    """
    groups = []
    tok = 0
    for g in range(N_GROUPS):
        tiles = list(range(g * T_MERGE, min((g + 1) * T_MERGE, N_TILES)))
        grp = {"tiles": tiles, "calls": [], "chunk_base": tok // TILE}
        for s in range(N_SEC):
            K = int(sum(cnt_max[t][s] for t in tiles))
            if K == 0:
                continue
            call = {"s": s, "K": K, "token_base": tok, "secs": []}
            off = 0
            for t in tiles:
                n_ch = cnt_max[t][s] // TILE
                if n_ch:
                    call["secs"].append((t, n_ch, off))
                off += n_ch
            grp["calls"].append(call)
            tok += K
        grp["n_chunks"] = tok // TILE - grp["chunk_base"]
        groups.append(grp)
    return groups, tok


def _prepare(x, edge_index):
    n = x.shape[0]
    assert n == N_NODES and x.shape[1] == 256

    src = np.ascontiguousarray(edge_index[0]).astype(np.int64)
    dst = np.ascontiguousarray(edge_index[1]).astype(np.int64)
    deg = (np.bincount(dst, minlength=n) + 1.0).astype(np.float64)
    dinv = (1.0 / np.sqrt(deg)).astype(np.float32)

    loops = np.arange(n, dtype=np.int64)
    src_a = np.concatenate([src, loops])
    dst_a = np.concatenate([dst, loops])

    # padded table row of each source + section bucketing
    srow = (src_a // SH) * SH_PAD + (src_a % SH)
    sec_a = srow // SEC_ROWS
    sid_a = (srow % SEC_ROWS).astype(np.int16)

    core_of = dst_a // SH
    per_core = []
    cnts = np.zeros((N_CORES, N_TILES, N_SEC), np.int64)
    for k in range(N_CORES):
        m = core_of == k
        d = dst_a[m] - k * SH
        s, sid = sec_a[m], sid_a[m]
        t = d >> 7
        order = np.lexsort((sid, s, t))
        d, s, sid, t = d[order], s[order], sid[order], t[order]
        np.add.at(cnts[k], (t, s), 1)
        per_core.append((d, s, sid, t))

    cnt_max = cnts.max(axis=0)
    cnt_pad = ((cnt_max + TILE - 1) // TILE * TILE).astype(np.int64)
    groups, total_tok = _layout(cnt_pad)
    total_chunks = total_tok // TILE

    sec_base = np.zeros((N_TILES, N_SEC), np.int64)
    for grp in groups:
        for call in grp["calls"]:
            for (t, n_ch, off) in call["secs"]:
                sec_base[t][call["s"]] = call["token_base"] + off * TILE

    # tile-major chunk permutation for the dloc slab (one S build per tile)
    chunk_perm = []
    tile_base = np.zeros(N_TILES, np.int64)
    for t in range(N_TILES):
        tile_base[t] = len(chunk_perm)
        for s2 in range(N_SEC):
            cb = sec_base[t][s2] // TILE
            for c in range(int(cnt_pad[t][s2] // TILE)):
                chunk_perm.append(cb + c)
    chunk_perm = np.asarray(chunk_perm, np.int64)
    max_tile_ch = int(np.diff(np.append(tile_base, len(chunk_perm))).max())

    idx_cols = total_tok // 16
    core_inputs = []
    for k in range(N_CORES):
        d, s, sid, t = per_core[k]
        flat_cnt = cnts[k].reshape(-1)
        sec_start = np.concatenate([[0], np.cumsum(flat_cnt)])[:-1]
        sec_id = t * N_SEC + s
        rank = np.arange(len(d)) - sec_start[sec_id]
        dest = sec_base.reshape(-1)[sec_id] + rank

        tok_idx = np.full(total_tok, PAD_IDX, np.int16)
        tok_dloc = np.zeros(total_tok, np.float16)
        tok_idx[dest] = sid
        tok_dloc[dest] = (d & (TILE - 1)).astype(np.float16)

        idx_arr = np.zeros((128, idx_cols), np.int16)
        for grp in groups:
            for call in grp["calls"]:
                a, K = call["token_base"], call["K"]
                wrp = tok_idx[a:a + K].reshape(-1, 16).T
                idx_arr[:, a // 16:(a + K) // 16] = np.tile(wrp, (8, 1))
        dloc_arr = np.ascontiguousarray(
            tok_dloc.reshape(total_chunks, TILE).T[:, chunk_perm])

        xT = np.zeros((256, SH_PAD), np.float16)
        xT[:, :SH] = x[k * SH:(k + 1) * SH].T.astype(np.float16)
        dinvT = np.zeros((128, SH_PAD), np.float16)
        dinvT[:, :SH] = dinv[k * SH:(k + 1) * SH][None, :].astype(np.float16)
        core_inputs.append({"xT": xT, "idx": idx_arr, "dloc": dloc_arr,
                            "dinvT": dinvT})

    plan = {"groups": groups, "total_tok": total_tok,
            "total_chunks": total_chunks, "idx_cols": idx_cols,
            "tile_base": tile_base, "max_tile_ch": max_tile_ch}
    return core_inputs, plan


def _weight_inputs(inputs):
    smalls = np.zeros((128, 8), np.float32)
    smalls[:HID, 0] = inputs["bp"]
    smalls[:HID, 1] = inputs["gamma1"]
    smalls[:HID, 2] = inputs["beta1"]
    smalls[:HID, 3] = inputs["gamma2"]
    smalls[:HID, 4] = inputs["beta2"]
    smalls[:F3, 5] = inputs["b3"]
    smalls[0, 6] = inputs["bh"][0]
    smalls[:, 7] = 1e-5
    wp = np.asarray(inputs["Wp"], np.float16)
    return {
        "Wpa": np.ascontiguousarray(wp[:128]),
        "Wpb": np.ascontiguousarray(wp[128:256]),
        "W1": np.asarray(inputs["W1"], np.float16),
        "W2": np.asarray(inputs["W2"], np.float16),
        "W3": np.asarray(inputs["W3"], np.float16),
        "Wh": np.asarray(inputs["Wh"], np.float16),
        "smalls": smalls,
    }


# --------------------------------------------------------------------------
# device program
# --------------------------------------------------------------------------

def _apply_chunks():
    out = []
    c = 0
    while c < SH_PAD:
        w = min(APPLY_W, SH_PAD - c)
        out.append((c, w))
        c += w
    return out


def build_program(plan, scale_const):
    groups = plan["groups"]
    rg = [list(range(N_CORES))]
    inv_n = 1.0 / N_NODES

    nc = bacc.Bacc("TRN2", target_bir_lowering=False, debug=False,
                   num_devices=N_CORES, num_swdge_queues=4)

    # I/O
    xT_d = nc.dram_tensor("xT", [256, SH_PAD], F16, kind="ExternalInput")
    idx_d = nc.dram_tensor("idx", [128, plan["idx_cols"]], I16,
                           kind="ExternalInput")
    dloc_d = nc.dram_tensor("dloc", [128, plan["total_chunks"]], F16,
                            kind="ExternalInput")
    dinvT_d = nc.dram_tensor("dinvT", [128, SH_PAD], F16,
                             kind="ExternalInput")
    Wpa_d = nc.dram_tensor("Wpa", [128, HID], F16, kind="ExternalInput")
    Wpb_d = nc.dram_tensor("Wpb", [128, HID], F16, kind="ExternalInput")
    W1_d = nc.dram_tensor("W1", [HID, HID], F16, kind="ExternalInput")
    W2_d = nc.dram_tensor("W2", [HID, HID], F16, kind="ExternalInput")
    W3_d = nc.dram_tensor("W3", [HID, F3], F16, kind="ExternalInput")
    Wh_d = nc.dram_tensor("Wh", [F3, 1], F16, kind="ExternalInput")
    smalls_d = nc.dram_tensor("smalls", [128, 8], F32, kind="ExternalInput")
    out_d = nc.dram_tensor("out", [1, SH], F32, kind="ExternalOutput")

    # internal scratch
    hw_shard = [nc.dram_tensor(f"hws{i}", [SH_PAD, HID], F16)
                for i in range(3)]
    hw_full = [nc.dram_tensor(f"hwf{i}", [SH_PAD * N_CORES, HID], F16)
               for i in range(3)]
    stl_d = [nc.dram_tensor(f"stl{i}", [128, 2], F32) for i in range(2)]
    stg_d = [nc.dram_tensor(f"stg{i}", [128, 2], F32, addr_space="Shared")
             for i in range(2)]

    with tile.TileContext(nc) as tc:
        with (
            tc.tile_pool(name="const", bufs=1) as constp,
            tc.tile_pool(name="gbuf", bufs=6) as gpool,
            tc.tile_pool(name="big", bufs=3) as bigpool,
            tc.tile_pool(name="spool", bufs=2) as spool,
            tc.tile_pool(name="meta", bufs=2) as metapool,
            tc.tile_pool(name="small", bufs=4) as smallpool,
            tc.tile_pool(name="head", bufs=1) as headpool,
            tc.tile_pool(name="psA", bufs=2, space="PSUM") as psA,
            tc.tile_pool(name="psB", bufs=3, space="PSUM") as psB,
            tc.tile_pool(name="psT", bufs=2, space="PSUM") as psT,
            tc.tile_pool(name="psH", bufs=1, space="PSUM") as psH,
        ):
            nc.gpsimd.load_library(library_config.mlp)

            # ---- constants ----
            iota = constp.tile([128, 128], F32, tag="iota", name="iota")
            nc.gpsimd.iota(iota[:], pattern=[[1, 128]], base=0,
                           channel_multiplier=0,
                           allow_small_or_imprecise_dtypes=True)
            iota_c = constp.tile([128, 1], F32, tag="iota_c", name="iota_c")
            nc.gpsimd.iota(iota_c[:], pattern=[[0, 1]], base=0,
                           channel_multiplier=1,
                           allow_small_or_imprecise_dtypes=True)
            iota16 = constp.tile([128, 128], F16, tag="iota16", name="iota16")
            nc.scalar.copy(iota16[:], iota[:])
            eye16 = constp.tile([128, 128], F16, tag="eye16", name="eye16")
            nc.vector.tensor_scalar(eye16[:], iota[:], iota_c[:], None,
                                    ALU.is_equal)

            def load_const(name, dram, shape, dt):
                t = constp.tile(shape, dt, tag=name)
                nc.sync.dma_start(t[:], dram[:])
                return t

            Wpa = load_const("Wpa", Wpa_d, [128, HID], F16)
            Wpb = load_const("Wpb", Wpb_d, [128, HID], F16)
            W1 = load_const("W1", W1_d, [HID, HID], F16)
            W2 = load_const("W2", W2_d, [HID, HID], F16)
            W3 = load_const("W3", W3_d, [HID, F3], F16)
            Wh = load_const("Wh", Wh_d, [F3, 1], F16)
            smalls = load_const("smalls", smalls_d, [128, 8], F32)
            dinvT = load_const("dinvT", dinvT_d, [128, SH_PAD], F16)
            dlocs = load_const("dloc", dloc_d, [128, plan["total_chunks"]],
                               F16)

            # big SBUF slabs (persistent)
            aggs = constp.tile([128, SH_PAD], F16, tag="aggs", name="aggs")
            h1T = constp.tile([128, SH_PAD], F16, tag="h1T", name="h1T")

            stats_sum = [constp.tile([128, N_GROUPS], F32, tag=f"ssum{i}",
                                     name=f"ssum{i}") for i in (0, 1)]
            stats_sq = [constp.tile([128, N_GROUPS], F32, tag=f"ssq{i}",
                                    name=f"ssq{i}") for i in (0, 1)]

            # ---- helper: emit row-major fp16 table rows from T-land chunk --
            def emit_rows(src_tile, c0, w, shard):
                # src_tile: [128, w] fp16 T-land (features x nodes)
                for b0 in range(0, w, 512):
                    bw = min(512, w - b0)
                    nj = bw // 128
                    pst = psT.tile([128, 512], F16, tag="pst", name="pst")
                    for j in range(nj):
                        nc.tensor.transpose(
                            pst[:, j * 128:(j + 1) * 128],
                            src_tile[:, b0 + j * 128:b0 + (j + 1) * 128],
                            eye16[:])
                    rsb = bigpool.tile([128, 512], F16, tag="row", name="row")
                    nc.scalar.copy(rsb[:, :nj * 128], pst[:, :nj * 128])
                    r0 = c0 + b0
                    dview = shard[r0:r0 + nj * 128, :].rearrange(
                        "(j p) f -> p j f", p=128)
                    sview = rsb[:, :nj * 128].rearrange(
                        "p (j f) -> p j f", f=128)
                    nc.sync.dma_start(dview, sview)

            # ---- stage 0: projection + table 1 ----
            for (c0, w) in _apply_chunks():
                xa = bigpool.tile([128, APPLY_W], F16, tag="xa", name="xa")
                nc.sync.dma_start(xa[:, :w], xT_d[0:128, c0:c0 + w])
                xb = bigpool.tile([128, APPLY_W], F16, tag="xb", name="xb")
                nc.sync.dma_start(xb[:, :w], xT_d[128:256, c0:c0 + w])
                ps = psB.tile([128, APPLY_W], F32, tag="psb", name="psb")
                nc.tensor.matmul(ps[:, :w], Wpa[:], xa[:, :w],
                                 start=True, stop=False)
                nc.tensor.matmul(ps[:, :w], Wpb[:], xb[:, :w],
                                 start=False, stop=True)
                h0 = bigpool.tile([128, APPLY_W], F16, tag="hsb", name="hsb")
                nc.scalar.activation(h0[:, :w], ps[:, :w], AF.Relu,
                                     bias=smalls[:, 0:1])
                ht = bigpool.tile([128, APPLY_W], F16, tag="ht", name="ht")
                nc.vector.tensor_tensor(ht[:, :w], h0[:, :w],
                                        dinvT[:, c0:c0 + w], ALU.mult)
                ps2 = psB.tile([128, APPLY_W], F32, tag="psb", name="psb")
                nc.tensor.matmul(ps2[:, :w], W1[:], ht[:, :w],
                                 start=True, stop=True)
                hw1 = bigpool.tile([128, APPLY_W], F16, tag="hw", name="hw")
                nc.scalar.copy(hw1[:, :w], ps2[:, :w])
                emit_rows(hw1, c0, w, hw_shard[0])

            if ABLATE != "noag" and PH >= 2:
                nc.gpsimd.collective_compute(
                    "AllGather", ALU.bypass, replica_groups=rg,
                    ins=[hw_shard[0][:]], outs=[hw_full[0][:]])

            # ---- aggregation pass (layers 1..3) ----
            def agg_pass(li):
                table = hw_full[li - 1]
                for grp in groups:
                    if not grp["calls"]:
                        continue
                    ic0 = grp["calls"][0]["token_base"] // 16
                    icw = sum(c["K"] for c in grp["calls"]) // 16
                    islab = metapool.tile([128, max(icw, 1)], I16,
                                          tag="islab", name="islab")
                    nc.sync.dma_start(islab[:, :icw], idx_d[:, ic0:ic0 + icw])
                    gts = {}
                    for call in grp["calls"]:
                        K, s = call["K"], call["s"]
                        gt = gpool.tile([128, K // TILE, HID], F16,
                                        tag="g", name="g")
                        lo = (call["token_base"]
                              - grp["calls"][0]["token_base"]) // 16
                        assert K <= 8192, K
                        if ABLATE == "nogather":
                            gts[s] = (gt, call)
                            continue
                        nc.gpsimd.dma_gather(
                            gt[:], table[s * SEC_ROWS:(s + 1) * SEC_ROWS, :],
                            islab[:, lo:lo + K // 16],
                            K, K, HID, single_packet=False, queue_num=s)
                        gts[s] = (gt, call)
                    ps = psA.tile([HID, 512], F32, tag="psa",
                                  name="psa")
                    for ti, t in enumerate(grp["tiles"]):
                        chunks = []
                        for s in sorted(gts):
                            gt, call = gts[s]
                            for (tt, n_ch, off) in call["secs"]:
                                if tt != t:
                                    continue
                                for c in range(n_ch):
                                    chunks.append((gt, off + c))
                        if ABLATE == "nomm":
                            chunks = chunks[:1]
                        n_ch_t = len(chunks)
                        St = spool.tile([128, plan["max_tile_ch"] * 128],
                                        F16, tag="S", name="St")
                        b = int(plan["tile_base"][t])
                        dv = dlocs[:, b:b + n_ch_t]
                        dX = bass.AP(dv.tensor, dv.offset,
                                     [list(dv.ap[0]), list(dv.ap[1]),
                                      [0, 128]])
                        iv = iota16[:]
                        iR = bass.AP(iv.tensor, iv.offset,
                                     [list(iv.ap[0]), [0, n_ch_t],
                                      list(iv.ap[1])])
                        nc.vector.tensor_tensor(
                            St[:, :n_ch_t * 128].rearrange(
                                "p (c f) -> p c f", f=128),
                            dX, iR, ALU.is_equal)
                        pcol = ti * 128
                        for i, (gt, gc) in enumerate(chunks):
                            nc.tensor.matmul(
                                ps[:, pcol:pcol + 128], gt[:, gc, :],
                                St[:, i * 128:(i + 1) * 128],
                                start=(i == 0), stop=(i == n_ch_t - 1))
                    yield grp, ps

            # ---- layers 1 and 2 ----
            for li in (1, 2):
                if PH < (3 if li == 1 else 5):
                    break
                ssum, ssq = stats_sum[li - 1], stats_sq[li - 1]
                for grp, ps in agg_pass(li):
                    t0 = grp["tiles"][0]
                    w = len(grp["tiles"]) * 128
                    gi = t0 // T_MERGE
                    cols = aggs[:, t0 * 128:t0 * 128 + w]
                    nc.vector.tensor_tensor(
                        cols, ps[:, :w],
                        dinvT[:, t0 * 128:t0 * 128 + w], ALU.mult)
                    nc.vector.tensor_reduce(ssum[:, gi:gi + 1], cols,
                                            mybir.AxisListType.X, ALU.add)
                    sq = smallpool.tile([128, 512], F32, tag="sq", name="sq",
                                        bufs=2)
                    nc.scalar.square(sq[:, :w], cols)
                    nc.vector.tensor_reduce(ssq[:, gi:gi + 1], sq[:, :w],
                                            mybir.AxisListType.X, ALU.add)
                # stats -> AllReduce
                if PH < (4 if li == 1 else 6):
                    break
                st = smallpool.tile([128, 2], F32, tag="stp", name="stp")
                nc.vector.tensor_reduce(st[:, 0:1], ssum[:],
                                        mybir.AxisListType.X, ALU.add)
                nc.vector.tensor_reduce(st[:, 1:2], ssq[:],
                                        mybir.AxisListType.X, ALU.add)
                nc.sync.dma_start(stl_d[li - 1][:], st[:])
                if ABLATE != "noag":
                    nc.gpsimd.collective_compute(
                        "AllReduce", ALU.add, replica_groups=rg,
                        ins=[stl_d[li - 1][:]], outs=[stg_d[li - 1][:]])
                ssb = smallpool.tile([128, 2], F32, tag="ssb", name="ssb")
                nc.sync.dma_start(ssb[:], (stl_d if ABLATE == "noag" else
                                           stg_d)[li - 1][:])

                # BN scale/bias (f32), then cast fp16
                scr = smallpool.tile([128, 6], F32, tag="bnscr", name="bnscr")
                nc.vector.tensor_scalar(scr[:, 0:1], ssb[:, 0:1], inv_n,
                                        None, ALU.mult)              # mean
                nc.vector.tensor_scalar(scr[:, 1:2], ssb[:, 1:2], inv_n,
                                        None, ALU.mult)              # E[x^2]
                nc.vector.tensor_tensor(scr[:, 2:3], scr[:, 0:1],
                                        scr[:, 0:1], ALU.mult)       # mean^2
                nc.vector.tensor_tensor(scr[:, 3:4], scr[:, 1:2],
                                        scr[:, 2:3], ALU.subtract)   # var
                nc.scalar.activation(scr[:, 4:5], scr[:, 3:4], AF.Sqrt,
                                     bias=smalls[:, 7:8])            # std
                inv_t = smallpool.tile([128, 1], F32, tag="invs", name="invs")
                nc.vector.reciprocal(inv_t[:], scr[:, 4:5])
                gamma_ap = smalls[:, 2 * li - 1:2 * li]
                beta_ap = smalls[:, 2 * li:2 * li + 1]
                scale_f = smallpool.tile([128, 1], F32, tag="scf", name="scf")
                bias_f = smallpool.tile([128, 1], F32, tag="bif", name="bif")
                nc.vector.tensor_tensor(scale_f[:], inv_t[:], gamma_ap,
                                        ALU.mult)
                mb = smallpool.tile([128, 1], F32, tag="mb", name="mb")
                nc.vector.tensor_tensor(mb[:], scr[:, 0:1], scale_f[:],
                                        ALU.mult)
                nc.vector.tensor_tensor(bias_f[:], beta_ap, mb[:],
                                        ALU.subtract)

                # BN apply + next table build
                for (c0, w) in _apply_chunks():
                    if li == 1:
                        htg = h1T[:, c0:c0 + w]
                        nc.scalar.activation(htg, aggs[:, c0:c0 + w],
                                             AF.Relu, bias=bias_f[:],
                                             scale=scale_f[:])
                    else:
                        tmp = bigpool.tile([128, APPLY_W], F16, tag="hsb",
                                           name="hsb")
                        nc.scalar.activation(tmp[:, :w], aggs[:, c0:c0 + w],
                                             AF.Relu, bias=bias_f[:],
                                             scale=scale_f[:])
                        htg = h1T[:, c0:c0 + w]
                        nc.vector.tensor_tensor(htg, tmp[:, :w], htg,
                                                ALU.add)
                    ht = bigpool.tile([128, APPLY_W], F16, tag="ht",
                                      name="ht")
                    nc.vector.tensor_tensor(ht[:, :w], htg,
                                            dinvT[:, c0:c0 + w], ALU.mult)
                    if li == 1:
                        ps2 = psB.tile([128, APPLY_W], F32, tag="psb",
                                       name="psb")
                        nc.tensor.matmul(ps2[:, :w], W2[:], ht[:, :w],
                                         start=True, stop=True)
                        hw2 = bigpool.tile([128, APPLY_W], F16, tag="hw",
                                           name="hw")
                        nc.scalar.copy(hw2[:, :w], ps2[:, :w])
                        emit_rows(hw2, c0, w, hw_shard[1])
                    else:
                        emit_rows(ht, c0, w, hw_shard[2])
                if ABLATE != "noag":
                    nc.gpsimd.collective_compute(
                        "AllGather", ALU.bypass, replica_groups=rg,
                        ins=[hw_shard[li][:]], outs=[hw_full[li][:]])

            # ---- layer 3 + head ----
            for grp, ps in (agg_pass(3) if PH >= 7 else ()):
                t0 = grp["tiles"][0]
                w = len(grp["tiles"]) * 128
                cols = aggs[:, t0 * 128:t0 * 128 + w]
                nc.vector.tensor_tensor(
                    cols, ps[:, :w],
                    dinvT[:, t0 * 128:t0 * 128 + w], ALU.mult)
            b3_ap = smalls[:F3, 5:6]
            bh_ap = smalls[0:1, 6:7]
            for (c0, w) in (_apply_chunks() if PH >= 8 else ()):
                ps3 = psB.tile([F3, APPLY_W], F32, tag="psb", name="ps3")
                nc.tensor.matmul(ps3[:, :w], W3[:], aggs[:, c0:c0 + w],
                                 start=True, stop=True)
                h3 = bigpool.tile([F3, APPLY_W], F16, tag="h3", name="h3")
                nc.scalar.activation(h3[:, :w], ps3[:, :w], AF.Relu,
                                     bias=b3_ap)
                psh = psH.tile([1, APPLY_W], F32, tag="psh", name="psh")
                nc.tensor.matmul(psh[:, :w], Wh[:], h3[:, :w],
                                 start=True, stop=True)
                th = headpool.tile([1, APPLY_W], F32, tag="th", name="th")
                nc.scalar.activation(th[:, :w], psh[:, :w], AF.Tanh,
                                     bias=bh_ap)
                ow = headpool.tile([1, APPLY_W], F32, tag="ow", name="ow")
                nc.scalar.mul(ow[:, :w], th[:, :w], float(scale_const))
                w_out = min(w, SH - c0)
                if w_out > 0:
                    nc.sync.dma_start(out_d[0:1, c0:c0 + w_out],
                                      ow[0:1, :w_out])
            if PH < 8:
                nc.sync.dma_start(out_d[0:1, 0:8], smalls[0:1, :])

    nc.compile()
    return nc


# --------------------------------------------------------------------------
# entry point
# --------------------------------------------------------------------------

_CACHE = {}


def _build_all(inputs, n_cores=8):
    x = np.asarray(inputs["x"], np.float32)
    ei = np.asarray(inputs["edge_index"])
    core_inputs, plan = _prepare(x, ei)
    wmap = _weight_inputs(inputs)
    in_maps = [{**ci, **wmap} for ci in core_inputs]
    key = (x.shape, ei.shape, float(np.asarray(inputs["scale"])))
    if key in _CACHE:
        nc = _CACHE[key]
    else:
        nc = build_program(plan, float(np.asarray(inputs["scale"])))
        _CACHE[key] = nc
    return nc, in_maps, plan


def kernel(**inputs) -> np.ndarray:
    nc, in_maps, plan = _build_all(inputs, N_CORES)
    res = run_bass_kernel_spmd(nc, in_maps, list(range(N_CORES)))
    outs = [res.results[k]["out"].reshape(-1) for k in range(N_CORES)]
    return np.concatenate(outs).reshape(-1, 1).astype(np.float32)
